# revision 7
# baseline (speedup 1.0000x reference)
"""Trainium2 Bass kernel: masked-bilinear channel-mixing Conv2d.

reference math (N=4, C=96, H=W=32, O=96, K=3, PAD=1):
    p = avgpool3x3(x, count_include_pad) -> [N, C, H, W] -> [N, L=1024, C]
    wm = weight * mask                              [O, C, C]
    y[n,l,o] = sum_{c,d} wm[o,c,d] p[n,l,c] p[n,l,d] + bias[o]

Sharding: data-parallel over the 4096 spatial locations -> 8 cores, each
takes half of one image (16 rows = 512 locations) and computes all 96
output channels. Weight/mask are replicated (host pre-transposes them to
c-major so each DMA line is contiguous); the avg-pool 1/81 factor and the
weight*mask product are computed on device.

Per-core device pipeline:
  pooling (4 DVE adds) -> PT [96(c), 512(loc)]
  wm = (wt/81)*mt      (one scalar_tensor_tensor)    [96(c), 9216(o,d)]
  per o: T_o = matmul(lhsT=wm[:, o], rhs=PT)         [96(d), 512] PSUM
         z_o = T_o * PT          (DVE; channel partitions align, no bcast)
         y += onehot_col(o)^T @ z_o  (matmul scatters sum_d into row o)
  y_sb = y + bias; DMA out.
"""
import numpy as np

import concourse.bass as bass
import concourse.bacc as bacc
import concourse.mybir as mybir
from concourse import tile
from concourse import bass_utils

C = 96
O = 96
HS = 16           # rows per core shard
W = 32
L = HS * W        # 512 locations per core
N_CORES = 8
F32 = mybir.dt.float32
F32R = mybir.dt.float32r


def _build_kernel(nc: bass.Bass):
    xs_d = nc.dram_tensor("xs", [C, 18 * 34], F32, kind="ExternalInput")
    wmcat_d = nc.dram_tensor("wmcat", [C, 2 * O * C], F32, kind="ExternalInput")
    b_d = nc.dram_tensor("bias", [O, 1], F32, kind="ExternalInput")
    y_d = nc.dram_tensor("y", [O, L], F32, kind="ExternalOutput")

    with tile.TileContext(nc) as tc:
        with (
            tc.tile_pool(name="const", bufs=1) as cpool,
            tc.tile_pool(name="work", bufs=1) as wpool,
            tc.tile_pool(name="z", bufs=6) as zpool,
            tc.tile_pool(name="tpsum", bufs=4, space="PSUM") as tpsum,
            tc.tile_pool(name="ypsum", bufs=1, space="PSUM") as ypsum,
        ):
            xs = cpool.tile([C, 18 * 34], F32)
            wmcat = cpool.tile([C, 2 * O * C], F32)
            wm = cpool.tile([C, O * C], F32R)
            bias = cpool.tile([O, 1], F32)
            # zo[:, 95] is ones, all else zero; zo[:, 95-o : 191-o] is a
            # [96, 96] matrix whose column o is ones -> as matmul lhsT it
            # scatters the partition-sum of rhs into row o of the output.
            zo = cpool.tile([C, 2 * O - 1], F32R)
            zof = cpool.tile([C, 2 * O - 1], F32)
            nc.sync.dma_start(xs[:], xs_d.ap())
            nc.sync.dma_start(bias[:], b_d.ap())
            # weight/mask host-packed per block: [blk] = [wt_blk | mt_blk],
            # so each wm block waits on exactly one DMA-queue semaphore and
            # the multiplies overlap the transfers.
            NBLK = 8
            BL = O * C // NBLK
            for blk in range(NBLK):
                base = blk * 2 * BL
                nc.sync.dma_start(wmcat[:, base:base + 2 * BL],
                                  wmcat_d.ap()[:, base:base + 2 * BL])
                nc.vector.tensor_mul(
                    wm[:, blk * BL:(blk + 1) * BL],
                    wmcat[:, base:base + BL],
                    wmcat[:, base + BL:base + 2 * BL])
            nc.vector.memset(zof[:], 0.0)
            nc.vector.memset(zof[:, O - 1:O], 1.0)
            nc.vector.tensor_scalar_mul(zo[:], zof[:], 1.0)

            # --- pooling: horizontal then vertical 3-tap box sums ---
            s1 = wpool.tile([C, 18 * 32], F32)
            s2 = wpool.tile([C, 18 * 32], F32)
            pt_raw = wpool.tile([C, L], F32)
            pt = wpool.tile([C, L], F32R)
            pt2 = wpool.tile([C, L], F32)
            x3 = xs[:].rearrange("c (h w) -> c h w", h=18)
            s1v = s1[:].rearrange("c (h w) -> c h w", h=18)
            s2v = s2[:].rearrange("c (h w) -> c h w", h=18)
            nc.vector.tensor_add(s1v, x3[:, :, 0:32], x3[:, :, 1:33])
            nc.vector.tensor_add(s2v, s1v, x3[:, :, 2:34])
            ptv = pt_raw[:].rearrange("c (h w) -> c h w", h=HS)
            pt2v = pt2[:].rearrange("c (h w) -> c h w", h=HS)
            nc.vector.tensor_add(pt2v, s2v[:, 0:16, :], s2v[:, 1:17, :])
            nc.vector.tensor_add(ptv, pt2v, s2v[:, 2:18, :])
            # p = boxsum/9; p enters the quadratic form twice -> 1/81 total
            nc.vector.tensor_scalar_mul(pt[:], pt_raw[:], 1.0 / 9.0)


            y_ps = ypsum.tile([O, L], F32)
            for o in range(O):
                t_ps = tpsum.tile([C, L], F32)
                nc.tensor.matmul(
                    t_ps[:], wm[:, o * C:(o + 1) * C], pt[:],
                    start=True, stop=True,
                )
                z = zpool.tile([C, L], F32R)
                nc.vector.tensor_mul(z[:], t_ps[:], pt[:])
                nc.tensor.matmul(
                    y_ps[:], zo[:, O - 1 - o:2 * O - 1 - o], z[:],
                    start=(o == 0), stop=(o == O - 1),
                )

            y_sb = wpool.tile([O, L], F32)
            nc.vector.tensor_scalar_add(y_sb[:], y_ps[:], bias[:])
            nc.sync.dma_start(y_d.ap(), y_sb[:])
    return nc


_NC_CACHE = {}


def _get_nc():
    if "nc" not in _NC_CACHE:
        nc = bacc.Bacc("TRN2", target_bir_lowering=False, debug=False,
                       enable_asserts=False)
        _build_kernel(nc)
        nc.compile()
        _NC_CACHE["nc"] = nc
    return _NC_CACHE["nc"]


def _prep_shards(x, weight, mask, bias):
    xpad = np.pad(np.asarray(x, np.float32), ((0, 0), (0, 0), (1, 1), (1, 1)))
    wt = np.asarray(weight, np.float32).transpose(1, 0, 2).reshape(C, O * C)
    mt = np.asarray(mask, np.float32).transpose(1, 0, 2).reshape(C, O * C)
    NBLK = 8
    BL = O * C // NBLK
    wmcat = np.empty((C, NBLK, 2, BL), np.float32)
    wmcat[:, :, 0, :] = wt.reshape(C, NBLK, BL)
    wmcat[:, :, 1, :] = mt.reshape(C, NBLK, BL)
    wmcat = np.ascontiguousarray(wmcat.reshape(C, 2 * O * C))
    b = np.ascontiguousarray(np.asarray(bias, np.float32).reshape(O, 1))
    in_maps = []
    for core in range(N_CORES):
        n, half = core // 2, core % 2
        h0 = half * HS
        xs = np.ascontiguousarray(
            xpad[n, :, h0:h0 + 18, :].reshape(C, 18 * 34))
        in_maps.append({"xs": xs, "wmcat": wmcat, "bias": b})
    return in_maps


def run_sharded(x, weight, mask, bias, **run_kwargs):
    """Run on the 8 NeuronCores; returns (y_full, BassKernelResults)."""
    nc = _get_nc()
    in_maps = _prep_shards(x, weight, mask, bias)
    res = bass_utils.run_bass_kernel_spmd(
        nc, in_maps, core_ids=list(range(N_CORES)), **run_kwargs)
    n_img = np.asarray(x).shape[0]
    y = np.empty((n_img, O, 32, 32), dtype=np.float32)
    for core in range(N_CORES):
        n, half = core // 2, core % 2
        h0 = half * HS
        y[n, :, h0:h0 + HS, :] = res.results[core]["y"].reshape(O, HS, W)
    return y, res


def kernel(x, weight, mask, bias):
    y, _ = run_sharded(x, weight, mask, bias)
    return y
